# revision 11
# baseline (speedup 1.0000x reference)
"""Causal self-attention (B=4, T=2048, D=1024, H=16, head_dim=64) on 8 TRN2
NeuronCores — v3: attention for pair p overlapped with q/k projection for
pair p+1; multiplicative causal masks on GpSimd; decoupled normalization.

Sharding: core c handles batch b = c//2 and head-half hh = c%2 (8 heads);
host sums the two partial output projections per batch.
"""
import os
import sys

sys.path.insert(0, "/opt/trn_rl_repo")

import numpy as np

import concourse.bass as bass
import concourse.mybir as mybir
import concourse.tile as tile
from concourse import bacc
from concourse.bass_utils import run_bass_kernel_spmd

F32 = mybir.dt.float32
F32R = mybir.dt.float32r
EXP = mybir.ActivationFunctionType.Exp
MUL = mybir.AluOpType.mult

B, T, DIM, HEADS, HD = 4, 2048, 1024, 16, 64
THETA = 10000.0
NCORES = 8


def _consts():
    freqs = 1.0 / THETA ** (np.arange(0, HD, 2, dtype=np.float32) / HD)
    t = np.arange(T, dtype=np.float32)
    ang = t[None, :] * freqs[np.arange(128) % 32, None]
    cosT = np.cos(ang).astype(np.float32)
    sinT = np.sin(ang).astype(np.float32)

    P = np.zeros((128, 128), dtype=np.float32)
    for i in range(128):
        base, il = (i // 64) * 64, i % 64
        if il < 32:
            P[i, base + il + 32] = -1.0
        else:
            P[i, base + il - 32] = 1.0
    PT = P.T.copy()

    k = np.arange(128)[:, None]
    q = np.arange(128)[None, :]
    tri01 = (k <= q).astype(np.float32)
    tri01j3 = np.zeros((128, 256), dtype=np.float32)
    tri01j3[:, 128:] = tri01
    return cosT, sinT, PT, tri01, tri01j3


def _build(repeat=1):
    nc = bacc.Bacc("TRN2", target_bir_lowering=False, debug=False)

    xT = nc.dram_tensor("xT", [DIM, T], F32R, kind="ExternalInput")
    wqk = nc.dram_tensor("wqk", [DIM, 1024], F32R, kind="ExternalInput")
    wv = nc.dram_tensor("wv", [DIM, 512], F32R, kind="ExternalInput")
    wo = nc.dram_tensor("wo", [512, DIM], F32R, kind="ExternalInput")
    cosT_d = nc.dram_tensor("cosT", [128, T], F32, kind="ExternalInput")
    sinT_d = nc.dram_tensor("sinT", [128, T], F32, kind="ExternalInput")
    PT_d = nc.dram_tensor("PT", [128, 128], F32R, kind="ExternalInput")
    tri_d = nc.dram_tensor("tri", [128, 128], F32R, kind="ExternalInput")
    trij3_d = nc.dram_tensor("trij3", [128, 256], F32R, kind="ExternalInput")
    ones_d = nc.dram_tensor("ones", [128, 128], F32R, kind="ExternalInput")
    outp = nc.dram_tensor("outp", [T, DIM], F32, kind="ExternalOutput")

    xr = xT.rearrange("(c p) t -> p c t", p=128)
    wqkr = wqk.rearrange("(c p) m -> p c m", p=128)

    with tile.TileContext(nc) as tc:
      for _rep in range(repeat):
        with (
            tc.tile_pool(name="glob", bufs=1) as glob,
            tc.tile_pool(name="qk", bufs=2) as qkpool,
            tc.tile_pool(name="p2", bufs=1) as p2,
            tc.tile_pool(name="p2st", bufs=2, space="PSUM") as p2st,
            tc.tile_pool(name="p2o", bufs=1, space="PSUM") as ps_out,
            tc.tile_pool(name="p2dram", bufs=8, space="DRAM") as p2dram,
        ):
          att = p2.tile([128, 4, T], F32R)  # att_norm^T [attdim, t]
          with (
            tc.tile_pool(name="xpool", bufs=2) as xpool,
            tc.tile_pool(name="wqkp", bufs=2) as wqkp,
            tc.tile_pool(name="p1t", bufs=2) as p1t,
            tc.tile_pool(name="p2pt", bufs=2) as p2pt,
            tc.tile_pool(name="p2u", bufs=2) as p2u,
            tc.tile_pool(name="p2n", bufs=2) as p2n,
            tc.tile_pool(name="p1ps", bufs=1, space="PSUM") as p1ps,
            tc.tile_pool(name="p1rot", bufs=1, space="PSUM") as p1rot,
          ):
            v_aug = glob.tile([128, 16, 8, 65], F32R)
            tri_sb = glob.tile([128, 128], F32R)
            trij3_sb = glob.tile([128, 256], F32R)
            cos_sb = glob.tile([128, T], F32)
            sin_sb = glob.tile([128, T], F32)
            PT_sb = glob.tile([128, 128], F32R)
            nc.sync.dma_start(out=tri_sb, in_=tri_d[:])
            nc.sync.dma_start(out=trij3_sb, in_=trij3_d[:])
            nc.sync.dma_start(out=cos_sb, in_=cosT_d[:])
            nc.sync.dma_start(out=sin_sb, in_=sinT_d[:])
            nc.sync.dma_start(out=PT_sb, in_=PT_d[:])
            nc.sync.dma_start(
                out=v_aug[:, :, :, 64:65],
                in_=ones_d.rearrange("p (a b o) -> p a b o", a=16, o=1),
            )

            qk_tiles = {}  # pair -> (q_tile, k_tile), each [128, T] f32r

            def proj_unit(m, n, x_t, wqk_m):
                """q (m<4) / k (m>=4) projection rows 128m, T-block n, + rope."""
                pair = m % 4
                dest = qk_tiles[pair][0 if m < 4 else 1]
                ncol = slice(n * 512, (n + 1) * 512)
                ps = p1ps.tile([128, 512], F32, tag="proj")
                for k in range(8):
                    nc.tensor.matmul(
                        ps[:], wqk_m[:, k, :], x_t[:, k, :],
                        start=(k == 0), stop=(k == 7),
                    )
                raw = p1t.tile([128, 512], F32R, tag="raw")
                nc.vector.tensor_copy(raw[:], ps[:])
                rotp = p1rot.tile([128, 512], F32)
                nc.tensor.matmul(rotp[:], PT_sb[:], raw[:], start=True, stop=True)
                t2 = p1t.tile([128, 512], F32, tag="t2")
                nc.vector.tensor_mul(t2[:], rotp[:], sin_sb[:, ncol])
                nc.vector.tensor_mul(dest[:, ncol], raw[:], cos_sb[:, ncol])
                nc.vector.tensor_add(dest[:, ncol], dest[:, ncol], t2[:])

            def qk_stream(pair):
                """Closures: allocate pair's q/k tiles and emit its 8 units."""
                ctx = {}
                out = []

                def alloc():
                    qt = qkpool.tile([128, T], F32R, tag="q", name=f"qt{pair}")
                    kt = qkpool.tile([128, T], F32R, tag="k", name=f"kt{pair}")
                    qk_tiles[pair] = (qt, kt)
                out.append(alloc)
                for n in range(4):
                    def load_x(n=n):
                        x_t = xpool.tile([128, 8, 512], F32R)
                        nc.sync.dma_start(out=x_t,
                                          in_=xr[:, :, n * 512:(n + 1) * 512])
                        ctx["x"] = x_t
                    out.append(load_x)
                    for m in (pair, pair + 4):
                        def unit(m=m, n=n):
                            wqk_m = wqkp.tile([128, 8, 128], F32R)
                            nc.sync.dma_start(
                                out=wqk_m, in_=wqkr[:, :, m * 128:(m + 1) * 128])
                            proj_unit(m, n, ctx["x"], wqk_m)
                        out.append(unit)
                return out

            # ---- Phase A: v projection (all T) + pair 0 q/k ----
            with tc.tile_pool(name="wvp", bufs=1) as wvp:
                wv_sb = wvp.tile([128, 8, 512], F32R)
                nc.sync.dma_start(out=wv_sb,
                                  in_=wv.rearrange("(c p) m -> p c m", p=128))
                qk_tiles[0] = (qkpool.tile([128, T], F32R, tag="q", name="qt0"),
                               qkpool.tile([128, T], F32R, tag="k", name="kt0"))
                for n in range(4):
                    x_t = xpool.tile([128, 8, 512], F32R)
                    nc.sync.dma_start(out=x_t, in_=xr[:, :, n * 512:(n + 1) * 512])
                    for ts in range(4):
                        psv = p1ps.tile([128, 512], F32, tag="proj")
                        for k in range(8):
                            nc.tensor.matmul(
                                psv[:], x_t[:, k, ts * 128:(ts + 1) * 128],
                                wv_sb[:, k, :], start=(k == 0), stop=(k == 7),
                            )
                        nc.vector.tensor_copy(
                            v_aug[:, n * 4 + ts, :, 0:64],
                            psv.rearrange("p (h d) -> p h d", h=8),
                        )
                    for m in (0, 4):
                        wqk_m = wqkp.tile([128, 8, 128], F32R)
                        nc.sync.dma_start(
                            out=wqk_m, in_=wqkr[:, :, m * 128:(m + 1) * 128])
                        proj_unit(m, n, x_t, wqk_m)

            # ---- Phase B: attention(p) interleaved with projections(p+1) ----
            if True:
                def kc_block(p, qb, kc, nkc, o2):
                    qt, kt = qk_tiles[p]
                    j = kc - 4 * qb
                    c0 = 0 if j < 0 else (256 if j == 3 else 128 * j)
                    qcol = slice(qb * 512 + c0, (qb + 1) * 512)
                    kcol = slice(kc * 128, (kc + 1) * 128)
                    st = p2st.tile([128, 1024], F32, tag="st")
                    nc.tensor.matmul(st[:, c0:512], kt[0:64, kcol],
                                     qt[0:64, qcol], start=True, stop=True)
                    nc.tensor.matmul(st[:, 512 + c0:1024], kt[64:128, kcol],
                                     qt[64:128, qcol], start=True, stop=True)
                    pt = p2pt.tile([128, 1024], F32R)
                    nc.scalar.activation(pt[:, c0:1024], st[:, c0:1024], EXP,
                                         bias=0.0, scale=0.125)
                    if j >= 0:
                        if j < 3:
                            ms = slice(128 * j, 128 * j + 128)
                            mk = tri_sb
                        else:
                            ms = slice(256, 512)
                            mk = trij3_sb
                        nc.gpsimd.tensor_tensor(pt[:, ms], pt[:, ms], mk[:], MUL)
                        ms2 = slice(512 + ms.start, 512 + ms.stop)
                        nc.gpsimd.tensor_tensor(pt[:, ms2], pt[:, ms2], mk[:], MUL)
                    nc.tensor.matmul(
                        o2[:, c0:512], v_aug[:, kc, 2 * p, :], pt[:, c0:512],
                        start=(kc == 0), stop=(kc == nkc - 1),
                        skip_group_check=True,
                    )
                    nc.tensor.matmul(
                        o2[:, 512 + c0:1024], v_aug[:, kc, 2 * p + 1, :],
                        pt[:, 512 + c0:1024], start=(kc == 0),
                        stop=(kc == nkc - 1), skip_group_check=True,
                    )

                def att_stream(p):
                    ctx = {}
                    out = []
                    for qb in range(4):
                        nkc = 4 * qb + 4

                        def mk_o2():
                            ctx["o2"] = ps_out.tile([65, 1024], F32, tag="o", name="o2")
                        out.append(mk_o2)
                        for kc in range(nkc):
                            out.append(lambda p=p, qb=qb, kc=kc, nkc=nkc:
                                       kc_block(p, qb, kc, nkc, ctx["o2"]))

                        def norm(p=p, qb=qb):
                            # decouple: copy unnormalized out + sums to SBUF,
                            # freeing the PSUM accumulator quickly
                            o2 = ctx["o2"]
                            attu = p2u.tile([65, 1024], F32, tag="attu")
                            nc.vector.tensor_copy(attu[:], o2[:])
                            rsum = p2n.tile([1, 1024], F32, tag="rsum", bufs=1)
                            nc.vector.reciprocal(rsum[:], attu[64:65, :])
                            scr = p2dram.tile([1, 1024], F32)
                            nc.sync.dma_start(out=scr[:], in_=rsum[:])
                            rbc = p2n.tile([64, 1024], F32, tag="rbc")
                            sap = scr[:]
                            nc.sync.dma_start(
                                out=rbc[:],
                                in_=bass.AP(tensor=sap.tensor, offset=sap.offset,
                                            ap=[[0, 64], [1, 1024]]),
                            )
                            qcols = slice(qb * 512, (qb + 1) * 512)
                            nc.vector.tensor_mul(att[0:64, p, qcols],
                                                 attu[0:64, 0:512], rbc[:, 0:512])
                            nc.vector.tensor_mul(att[64:128, p, qcols],
                                                 attu[0:64, 512:1024],
                                                 rbc[:, 512:1024])
                        out.append(norm)
                    return out

                for p in range(4):
                    a_stream = att_stream(p)
                    q_stream = qk_stream(p + 1) if p < 3 else []
                    na, nq = len(a_stream), len(q_stream)
                    qi = 0
                    for i, fn in enumerate(a_stream):
                        fn()
                        want = ((i + 1) * nq) // na
                        while qi < want:
                            q_stream[qi]()
                            qi += 1

          # ---- Phase C: output projection ----
          if True:
                with (tc.tile_pool(name="p3", bufs=1) as p3,
                      tc.tile_pool(name="p3ob", bufs=2) as p3ob):
                    wo_sb = p3.tile([128, 4, 1024], F32R)
                    nc.sync.dma_start(out=wo_sb,
                                      in_=wo.rearrange("(c p) m -> p c m", p=128))
                    for tcb in range(16):
                        for od in range(2):
                            po = p2st.tile([128, 512], F32, tag="st")
                            for ac in range(4):
                                nc.tensor.matmul(
                                    po[:], att[:, ac, tcb * 128:(tcb + 1) * 128],
                                    wo_sb[:, ac, od * 512:(od + 1) * 512],
                                    start=(ac == 0), stop=(ac == 3),
                                )
                            ob = p3ob.tile([128, 512], F32, tag="ob")
                            nc.vector.tensor_copy(ob[:], po[:])
                            nc.sync.dma_start(
                                out=outp[tcb * 128:(tcb + 1) * 128,
                                         od * 512:(od + 1) * 512],
                                in_=ob[:],
                            )
    nc.compile()
    return nc


_NC = {}


def _get_nc(repeat=1):
    if repeat not in _NC:
        _NC[repeat] = _build(repeat)
    return _NC[repeat]


def _in_maps(x, w_qkv, w_out):
    cosT, sinT, PT, tri01, tri01j3 = _consts()
    maps = []
    for c in range(NCORES):
        b, hh = c // 2, c % 2
        wqkm = np.ascontiguousarray(np.concatenate(
            [w_qkv[:, 512 * hh:512 * hh + 512],
             w_qkv[:, 1024 + 512 * hh:1024 + 512 * hh + 512]], axis=1))
        wvm = np.ascontiguousarray(w_qkv[:, 2048 + 512 * hh:2048 + 512 * hh + 512])
        wom = np.ascontiguousarray(w_out[512 * hh:512 * hh + 512, :])
        xTb = np.ascontiguousarray(x[b].T)
        maps.append(dict(xT=xTb, wqk=wqkm, wv=wvm, wo=wom, cosT=cosT,
                         sinT=sinT, PT=PT, tri=tri01, trij3=tri01j3,
                         ones=np.ones((128, 128), dtype=np.float32)))
    return maps


def kernel(x, w_qkv, w_out):
    x = np.ascontiguousarray(x, dtype=np.float32)
    w_qkv = np.ascontiguousarray(w_qkv, dtype=np.float32)
    w_out = np.ascontiguousarray(w_out, dtype=np.float32)

    nc = _get_nc(int(os.environ.get("KREPEAT", "1")))
    r = run_bass_kernel_spmd(nc, _in_maps(x, w_qkv, w_out),
                             core_ids=list(range(NCORES)))
    out = np.empty((B, T, DIM), dtype=np.float32)
    for b in range(B):
        out[b] = r.results[2 * b]["outp"] + r.results[2 * b + 1]["outp"]
    kernel.last_results = r
    return out
